# revision 1
# baseline (speedup 1.0000x reference)
"""Trainium2 Bass kernel for nn_Block_27212912788319 (dense transformer block).

Sharding: 8 NeuronCores = 2 batch groups (cores 0-3 -> batch 0, cores 4-7 ->
batch 1) x 4-way tensor parallel. TP rank r holds 4 attention heads (heads
sorted by ALiBi slope and dealt round-robin), 1/4 of the p (gated-MLP)
features, the matching rows of W_in and columns of W_out.

v2 schedule (vs baseline): per-block software pipeline keeps the PE queue
dense so HAM stays un-throttled, and hides all collectives:

  INIT+warmupAR, INLN, INP0, INP1, POST0+RS0, INP2, FIN0, POST1+RS1,
  INP3, FIN1, POST2+RS2, POST3+RS3, FIN2, FIN3

- A dummy tiny AllReduce fires at t~0 to absorb the one-time collective
  warmup (~85us barrier + ~70us first-op cost) under the input layernorm
  and first in_proj.
- Stats matmuls are delayed one M-chunk so the PE never waits on ACT/DVE.
- v's mid-layernorm is folded into the attention AV matmul: vhat carries
  raw v (later scaled by r_j in place), a ones column at row 64 (softmax
  z) and a c_j = m_j*r_j column at row 96 (mean correction), so
  o = (AV[0:64] - bcast(AV[96])) * bcast(1/z).
- The pg roll-shift runs on the PE (shift matrix + boundary-row
  accumulate) instead of partition-strided DMAs.
- ReduceScatter runs in bf16 (half the bytes).
"""

import os
import sys
from contextlib import ExitStack

# debug bisection: 1=inln, 2=+inproj/AR, 3=+attention, 4=+pg/outproj/RS,
# 5=full (default); 10=inln w/o warmup AR; 20=inproj w/o stats AR
STAGE = int(os.environ.get("KSTAGE", "5"))
NO_WARM = STAGE == 10
NO_STATS_AR = STAGE == 20
if STAGE == 10:
    STAGE = 1
if STAGE == 20:
    STAGE = 2

for _p in ("/opt/trn_rl_repo", "/root/.axon_site/_ro/trn_rl_repo"):
    if _p not in sys.path:
        sys.path.insert(0, _p)

import numpy as np
import ml_dtypes

import concourse.bass as bass
import concourse.bacc as bacc
import concourse.mybir as mybir
from concourse import tile
from concourse.bass_utils import run_bass_kernel_spmd

F32 = mybir.dt.float32
BF16 = mybir.dt.bfloat16
AF = mybir.ActivationFunctionType
ALU = mybir.AluOpType
AX = mybir.AxisListType

# ---------------- problem constants ----------------
B, H, D = 2, 16, 64
HID = H * D                  # 1024
EF = 4
QKVP = HID * (3 + EF)        # 7168
PFULL = HID * EF             # 4096
TP = 4
HPC = H // TP                # 4 heads per core
QW = HPC * D                 # 256
PW = PFULL // TP             # 1024
LOCF = 3 * QW + PW           # 1792
EPS = 1e-5
C_SAFE = 32.0
PRUNE_MARGIN = 92.0
NEG = -1e30

BLK = 512
JC = 128
NKC = HID // 128             # 8
NMC = LOCF // 128            # 14
OKC = (QW + PW) // 128       # 10
VST = 97                     # vhat chunk stride: 64 v + z@64 + pad + c@96
NCORES = 8
LAST_RESULT = None
REPLICA_GROUPS = [[0, 1, 2, 3], [4, 5, 6, 7]]


def _prune_dists(slopes_sorted_desc, L):
    d = []
    for s in range(HPC):
        smin = float(min(slopes_sorted_desc[TP * s: TP * s + TP]))
        if smin <= PRUNE_MARGIN / (L + JC):
            d.append(L + JC)
        else:
            d.append(int(np.ceil(PRUNE_MARGIN / smin)))
    return d


def _kept_chunks(dist, blk):
    i0 = blk * BLK
    njc = (i0 + BLK) // JC
    jc_min = max(0, -(-(i0 - (JC - 1) - dist) // JC))
    return list(range(jc_min, njc))


def build_program(L, prune_dists, identity_outln,
                  identity_inln=True, identity_midln=True):
    NBLK = L // BLK
    TOKC = L // 128
    NJ = L // JC
    nc = bacc.Bacc(None, target_bir_lowering=False)

    # ---------------- I/O ----------------
    x_in = nc.dram_tensor("x", [L, HID], F32, kind="ExternalInput")
    w_inT = nc.dram_tensor("w_inT", [HID, LOCF + 1], BF16, kind="ExternalInput")
    w_outT = nc.dram_tensor("w_outT", [QW + PW, HID], BF16, kind="ExternalInput")
    mid_g = nc.dram_tensor("mid_g", [LOCF + 1], F32, kind="ExternalInput")
    mid_b = nc.dram_tensor("mid_b", [LOCF + 1], F32, kind="ExternalInput")
    in_g = nc.dram_tensor("in_g", [HID], F32, kind="ExternalInput")
    in_b = nc.dram_tensor("in_b", [HID], F32, kind="ExternalInput")
    out_g = nc.dram_tensor("out_g", [HID], F32, kind="ExternalInput")
    out_b = nc.dram_tensor("out_b", [HID], F32, kind="ExternalInput")
    tal8_d = nc.dram_tensor("tal8", [HPC, JC, BLK], F32, kind="ExternalInput")
    tri8_d = nc.dram_tensor("tri8", [JC, JC], F32, kind="ExternalInput")
    ccb_d = nc.dram_tensor("ccb", [HPC, 128, NBLK * NJ], F32,
                           kind="ExternalInput")
    eyeb_d = nc.dram_tensor("eyeb", [2 * D, D], F32, kind="ExternalInput")
    id128_d = nc.dram_tensor("id128", [128, 128], BF16, kind="ExternalInput")
    ishb_d = nc.dram_tensor("ishb", [128, 128], BF16, kind="ExternalInput")
    eyef_d = nc.dram_tensor("eyef", [128, 128], F32, kind="ExternalInput")
    y_out = nc.dram_tensor("y", [L // TP, HID], F32, kind="ExternalOutput")

    # internal DRAM
    warm_in = nc.dram_tensor("warm_in", [1, 8], F32)
    warm_out = nc.dram_tensor("warm_out", [1, 8], F32)
    st_in = nc.dram_tensor("st_in", [NBLK, 1, 2 * BLK], F32)
    st_out = nc.dram_tensor("st_out", [NBLK, 1, 2 * BLK], F32)
    po_in = nc.dram_tensor("po_in", [NBLK, BLK, HID], BF16)
    po_out = nc.dram_tensor("po_out", [NBLK, JC, HID], BF16)

    ctx = ExitStack()
    with ctx:
        tc = ctx.enter_context(tile.TileContext(nc))

        # ---------------- persistent tiles ----------------
        pers = ctx.enter_context(tc.tile_pool(name="pers", bufs=1))
        xs = [pers.tile([128, L + 4], BF16, name=f"xs{c}") for c in range(NKC)]
        wi = [pers.tile([128, LOCF + 1], BF16, name=f"wi{c}") for c in range(NKC)]
        wo = [pers.tile([128, HID], BF16, name=f"wo{c}") for c in range(OKC)]
        hk = [pers.tile([128, L], BF16, name=f"hk{c}") for c in range(2)]
        vhat = [pers.tile([128, NJ * VST], BF16, name=f"vhat{s}")
                for s in range(HPC)]
        tal8 = [pers.tile([128, BLK], F32, name=f"tal8_{s}") for s in range(HPC)]
        tri8 = pers.tile([128, JC], F32, name="tri8")
        ccbs = [pers.tile([128, NBLK * NJ], F32, name=f"ccb{s}")
                for s in range(HPC)]
        eyeb_t = pers.tile([2 * D, D], F32, name="eyeb_t")
        id128_t = pers.tile([128, 128], BF16, name="id128_t")
        ishb_t = pers.tile([128, 128], BF16, name="ishb_t")
        eyef_t = pers.tile([128, 128], F32, name="eyef_t")
        onesb_t = pers.tile([128, 1], BF16, name="onesb_t")
        onesbf_t = pers.tile([128, 1], F32, name="onesbf_t")
        onesf_t = pers.tile([1, 128], F32, name="onesf_t")
        onesb2_t = pers.tile([1, 128], BF16, name="onesb2_t")
        eps_t = pers.tile([128, 1], F32, name="eps_t")
        scl8_t = pers.tile([128, 1], F32, name="scl8_t")
        wsb = pers.tile([1, 8], F32, name="wsb")
        if not identity_midln:
            midg_t = pers.tile([128, NMC + 1], F32, name="midg_t")
            midb_t = pers.tile([128, NMC + 1], F32, name="midb_t")
        if not identity_inln:
            ing_t = pers.tile([128, NKC], F32, name="ing_t")
            inb_t = pers.tile([128, NKC], F32, name="inb_t")
        if not identity_outln:
            og_row = pers.tile([1, HID], F32, name="og_row")
            ob_row = pers.tile([1, HID], F32, name="ob_row")
            og_bc = pers.tile([128, HID], F32, name="og_bc")
            ob_bc = pers.tile([128, HID], F32, name="ob_bc")

        # ---------------- PSUM pools ----------------
        pmm = ctx.enter_context(tc.tile_pool(name="pmm", bufs=3, space="PSUM"))
        pav_pool = ctx.enter_context(tc.tile_pool(name="pav", bufs=2,
                                                  space="PSUM"))
        pstat_pool = ctx.enter_context(tc.tile_pool(name="pstat", bufs=1,
                                                    space="PSUM"))

        # ---------------- warmup collective (absorbs comm init) --------
        if not NO_WARM:
            nc.vector.memset(wsb[:, :], 1.0)
            nc.sync.dma_start(warm_in[:, :], wsb[:, :])
            nc.gpsimd.collective_compute(
                "AllReduce", ALU.add, replica_groups=REPLICA_GROUPS,
                ins=[warm_in[:, :]], outs=[warm_out[:, :]])

        # ---------------- const DMAs ----------------
        for c in range(NKC):
            nc.sync.dma_start(wi[c][:, :], w_inT[128 * c:128 * (c + 1), :])
        for c in range(OKC):
            nc.sync.dma_start(wo[c][:, :], w_outT[128 * c:128 * (c + 1), :])
        for s in range(HPC):
            nc.sync.dma_start(tal8[s][:, :], tal8_d[s])
            nc.sync.dma_start(ccbs[s][:, :], ccb_d[s])
        nc.sync.dma_start(tri8[:, :], tri8_d[:, :])
        nc.sync.dma_start(eyeb_t[:, :], eyeb_d[:, :])
        nc.sync.dma_start(id128_t[:, :], id128_d[:, :])
        nc.sync.dma_start(ishb_t[:, :], ishb_d[:, :])
        nc.sync.dma_start(eyef_t[:, :], eyef_d[:, :])
        nc.vector.memset(onesb_t[:, :], 1.0)
        nc.vector.memset(onesbf_t[:, :], 1.0)
        nc.vector.memset(onesf_t[:, :], 1.0)
        nc.vector.memset(onesb2_t[:, :], 1.0)
        nc.vector.memset(eps_t[:, :], EPS)
        nc.vector.memset(scl8_t[:, :], 0.125)
        for s in range(HPC):
            nc.vector.memset(vhat[s][:, :], 1.0)
        nc.vector.memset(xs[0][:, 0:1], 0.0)
        nc.vector.memset(xs[1][:, 0:3], 0.0)
        if not identity_midln:
            for c in range(NMC):
                nc.sync.dma_start(midg_t[:, c:c + 1], mid_g[128 * c:128 * (c + 1)])
                nc.sync.dma_start(midb_t[:, c:c + 1], mid_b[128 * c:128 * (c + 1)])
            nc.sync.dma_start(midg_t[0:1, NMC:NMC + 1], mid_g[LOCF:LOCF + 1])
            nc.sync.dma_start(midb_t[0:1, NMC:NMC + 1], mid_b[LOCF:LOCF + 1])
        if not identity_inln:
            for c in range(NKC):
                nc.sync.dma_start(ing_t[:, c:c + 1], in_g[128 * c:128 * (c + 1)])
                nc.sync.dma_start(inb_t[:, c:c + 1], in_b[128 * c:128 * (c + 1)])
        if not identity_outln:
            nc.sync.dma_start(og_row[0:1, :], out_g[:])
            nc.sync.dma_start(ob_row[0:1, :], out_b[:])
            for half in range(HID // BLK):
                sl = slice(BLK * half, BLK * (half + 1))
                pg1 = pmm.tile([128, BLK], F32, tag="mm")
                nc.tensor.matmul(pg1[:, :], onesf_t[0:1, :], og_row[0:1, sl],
                                 start=True, stop=True)
                nc.vector.tensor_copy(og_bc[:, sl], pg1[:, :])
                pg2 = pmm.tile([128, BLK], F32, tag="mm")
                nc.tensor.matmul(pg2[:, :], onesf_t[0:1, :], ob_row[0:1, sl],
                                 start=True, stop=True)
                nc.vector.tensor_copy(ob_bc[:, sl], pg2[:, :])

        big = ctx.enter_context(tc.tile_pool(name="big", bufs=2))
        bp = ctx.enter_context(tc.tile_pool(name="bp", bufs=2))
        catp = ctx.enter_context(tc.tile_pool(name="catp", bufs=1))

        # row arenas: single-partition scratch packed into two wide tiles
        # (a [1, W] tile costs W bytes/partition, so per-row tiles are
        # wasteful; slices of one arena share the allocation)
        rowf = bp.tile([1, 2 * BLK], F32, tag="rowf", bufs=1, name="rowf")
        rowb = bp.tile([1, 18 * BLK], BF16, tag="rowb", bufs=1, name="rowb")

        def rowb_sl(i):
            return rowb[0:1, i * BLK:(i + 1) * BLK]

        def ln_rows_col(t, n_feat):
            """cols 0 (sum), 1 (sumsq) -> col 6 = r, col 7 = -m*r."""
            nc.vector.tensor_scalar_mul(t[:, 2:3], t[:, 0:1], 1.0 / n_feat)
            nc.vector.tensor_scalar_mul(t[:, 3:4], t[:, 1:2], 1.0 / n_feat)
            nc.vector.scalar_tensor_tensor(
                out=t[:, 4:5], in0=t[:, 2:3], scalar=-1.0,
                in1=t[:, 2:3], op0=ALU.mult, op1=ALU.mult)
            nc.vector.tensor_add(t[:, 4:5], t[:, 4:5], t[:, 3:4])
            nc.scalar.activation(t[:, 5:6], t[:, 4:5], AF.Sqrt,
                                 bias=eps_t[:, 0:1])
            nc.vector.reciprocal(t[:, 6:7], t[:, 5:6])
            nc.vector.scalar_tensor_tensor(
                out=t[:, 7:8], in0=t[:, 2:3], scalar=-1.0,
                in1=t[:, 6:7], op0=ALU.mult, op1=ALU.mult)

        # ---------------- input layernorm + transpose + shift ----------
        SHIFT = {0: 1, 1: 3}
        for t in range(TOKC):
            xt = big.tile([128, HID], F32, tag="bx")
            nc.sync.dma_start(xt[:, :], x_in[128 * t:128 * (t + 1), :])
            srow = big.tile([128, 8], F32, tag="bsrow")
            sqx = big.tile([128, HID], BF16, tag="bscr", bufs=1)
            nc.vector.tensor_reduce(srow[:, 0:1], xt[:, :], axis=AX.X,
                                    op=ALU.add)
            nc.vector.tensor_mul(sqx[:, :], xt[:, :], xt[:, :])
            nc.vector.tensor_reduce(srow[:, 1:2], sqx[:, :], axis=AX.X,
                                    op=ALU.add)
            ln_rows_col(srow, HID)
            xn = big.tile([128, HID], F32, tag="bxn", bufs=1)
            nc.vector.tensor_scalar(xn[:, :], xt[:, :], srow[:, 6:7],
                                    srow[:, 7:8], ALU.mult, ALU.add)
            for c in range(NKC):
                ptt = pmm.tile([128, BLK], F32, tag="mm")
                pt = ptt[:, 0:128]
                nc.tensor.transpose(pt, xn[:, 128 * c:128 * (c + 1)],
                                    eyef_t[:, :])
                d0 = 128 * t + SHIFT.get(c, 0)
                if identity_inln:
                    nc.vector.tensor_copy(xs[c][:, d0:d0 + 128], pt)
                else:
                    nc.vector.tensor_scalar(xs[c][:, d0:d0 + 128], pt,
                                            ing_t[:, c:c + 1],
                                            inb_t[:, c:c + 1],
                                            ALU.mult, ALU.add)

        # per-block state carried between phases
        blk_state = {}

        def emit_inp(b):
            """in_proj + relu + (delayed) stats matmuls + raw-v transposes
            + stats DMA + AllReduce."""
            i0 = b * BLK
            ps_sum = pstat_pool.tile([1, BLK], F32, tag="ssum")
            ps_sq = pstat_pool.tile([1, BLK], F32, tag="ssq")
            qraw = [bp.tile([128, BLK], BF16, tag=f"qr{c}", name=f"qr{c}_{b}")
                    for c in range(2)]
            vtmp = [bp.tile([128, BLK], F32, tag=f"vt{c}", name=f"vt{c}_{b}",
                            bufs=1)
                    for c in range(2)]
            praw = [bp.tile([128, BLK], BF16, tag=f"pr{c}", name=f"pr{c}_{b}")
                    for c in range(8)]
            pbord = rowb_sl(16 + b % 2)
            pending = None  # (ht_ap, h2t, mc)

            def flush_stats():
                nonlocal pending
                if pending is None:
                    return
                ht_ap, h2t, mc = pending
                ones_ap = onesbf_t[:, :] if 4 <= mc < 6 else onesb_t[:, :]
                nc.tensor.matmul(ps_sum[0:1, :], ones_ap, ht_ap,
                                 start=(mc == 0), stop=(mc == NMC - 1))
                nc.tensor.matmul(ps_sq[0:1, :], onesb_t[:, :], h2t[:, :],
                                 start=(mc == 0), stop=(mc == NMC - 1))
                pending = None

            for mc in range(NMC + 1):
                mw = 128 if mc < NMC else 1
                pm = pmm.tile([128, BLK], F32, tag="mm")
                for kc in range(NKC):
                    nc.tensor.matmul(
                        pm[0:mw, :],
                        wi[kc][:, 128 * mc:128 * mc + mw],
                        xs[kc][:, i0:i0 + BLK],
                        start=(kc == 0), stop=(kc == NKC - 1))
                flush_stats()
                if mc < 2:
                    dest_ap = qraw[mc][:, :]
                elif mc < 4:
                    dest_ap = hk[mc - 2][:, i0:i0 + BLK]
                elif mc < 6:
                    dest_ap = vtmp[mc - 4][:, :]
                elif mc < NMC:
                    dest_ap = praw[mc - 6][:, :]
                else:
                    dest_ap = pbord
                nc.scalar.activation(dest_ap, pm[0:mw, :], AF.Relu)
                if mc < NMC:
                    h2t = bp.tile([128, BLK], BF16, tag="h2")
                    nc.vector.tensor_mul(h2t[:, :], dest_ap, dest_ap)
                    pending = (dest_ap, h2t, mc)
            flush_stats()

            # raw v transposes into vhat (no AR dependency)
            for s in range(HPC):
                off = 64 * (s % 2)
                for u in range(BLK // JC):
                    jj = b * (BLK // JC) + u
                    ptv = pmm.tile([128, BLK], F32, tag="mm")
                    nc.tensor.transpose(
                        ptv[:, 0:D],
                        vtmp[s // 2][off:off + D, JC * u:JC * (u + 1)],
                        eyeb_t[off:off + D, :])
                    nc.vector.tensor_copy(
                        vhat[s][:, VST * jj:VST * jj + D], ptv[:, 0:D])

            # stats out + AllReduce
            stats_sb = bp.tile([1, 2 * BLK], F32, tag="stats", bufs=1)
            nc.vector.tensor_copy(stats_sb[0:1, 0:BLK], ps_sum[0:1, :])
            nc.vector.tensor_copy(stats_sb[0:1, BLK:2 * BLK], ps_sq[0:1, :])
            nc.sync.dma_start(st_in[b], stats_sb[0:1, :])
            if not NO_STATS_AR:
                nc.gpsimd.collective_compute(
                    "AllReduce", ALU.add, replica_groups=REPLICA_GROUPS,
                    ins=[st_in[b]], outs=[st_out[b]])
            blk_state[b] = dict(qraw=qraw, praw=praw, pbord=pbord)

        def emit_post(b):
            """mid-LN setup from AR result, q/k normalize, attention, pg,
            out_proj, ReduceScatter."""
            i0 = b * BLK
            st = blk_state.pop(b)
            qraw, praw, pbord = st["qraw"], st["praw"], st["pbord"]

            rows = bp.tile([1, 2 * BLK], F32, tag="rows", bufs=1)
            nc.sync.dma_start(rows[0:1, :], st_out[b])
            # stat columns: pst cols 0-3 sums, 8-11 sqs
            pst = pmm.tile([128, BLK], F32, tag="mm")
            for c in range(4):
                nc.tensor.transpose(pst[:, c:c + 1],
                                    rows[0:1, 128 * c:128 * (c + 1)],
                                    eyef_t[0:1, 0:1])
                nc.tensor.transpose(pst[:, 8 + c:9 + c],
                                    rows[0:1, BLK + 128 * c:BLK + 128 * (c + 1)],
                                    eyef_t[0:1, 0:1])
            stT = bp.tile([128, 24], F32, tag="stT", bufs=1)
            # mT = sums/QKVP ; e2 = sqs/QKVP ; var = e2 - mT^2
            nc.vector.tensor_scalar_mul(stT[:, 0:4], pst[:, 0:4], 1.0 / QKVP)
            nc.vector.tensor_scalar_mul(stT[:, 4:8], pst[:, 8:12], 1.0 / QKVP)
            nc.vector.tensor_mul(stT[:, 8:12], stT[:, 0:4], stT[:, 0:4])
            nc.vector.tensor_sub(stT[:, 4:8], stT[:, 4:8], stT[:, 8:12])
            nc.scalar.activation(stT[:, 8:12], stT[:, 4:8], AF.Sqrt,
                                 bias=eps_t[:, 0:1])
            nc.vector.reciprocal(stT[:, 12:16], stT[:, 8:12])
            nc.vector.tensor_mul(stT[:, 16:20], stT[:, 0:4], stT[:, 12:16])
            # r row via transpose-back; m row via 1-lane scale
            prow = pmm.tile([128, BLK], F32, tag="mm")
            for c in range(4):
                nc.tensor.transpose(prow[0:1, 128 * c:128 * (c + 1)],
                                    stT[:, 12 + c:13 + c], eyef_t[:, :])
            rrow = rowf[0:1, 0:BLK]
            nc.scalar.copy(rrow, prow[0:1, :])
            mrow = rowf[0:1, BLK:2 * BLK]
            nc.vector.tensor_scalar_mul(mrow, rows[0:1, 0:BLK], 1.0 / QKVP)
            # broadcast mb / rb
            pmb = pmm.tile([128, BLK], F32, tag="mm")
            nc.tensor.matmul(pmb[:, :], onesf_t[0:1, :], mrow,
                             start=True, stop=True)
            mb = bp.tile([128, BLK], BF16, tag="mb", bufs=1)
            nc.vector.tensor_copy(mb[:, :], pmb[:, :])
            prb = pmm.tile([128, BLK], F32, tag="mm")
            nc.tensor.matmul(prb[:, :], onesf_t[0:1, :], rrow,
                             start=True, stop=True)
            rb = bp.tile([128, BLK], BF16, tag="rb", bufs=1)
            nc.vector.tensor_copy(rb[:, :], prb[:, :])

            # vhat fixups: scale raw v cols by r_j, write c_j col
            for s in range(HPC):
                for u in range(BLK // JC):
                    jj = b * (BLK // JC) + u
                    nc.vector.tensor_scalar_mul(
                        vhat[s][:, VST * jj:VST * jj + D],
                        vhat[s][:, VST * jj:VST * jj + D],
                        stT[:, 12 + u:13 + u])
                    nc.vector.tensor_copy(
                        vhat[s][:, VST * jj + 96:VST * jj + 97],
                        stT[:, 16 + u:17 + u])

            def mid_norm(dst, srcap, col, mw=128):
                t1 = bp.tile([128, BLK], BF16, tag="t1")
                nc.vector.tensor_sub(t1[0:mw, :], srcap, mb[0:mw, :])
                if identity_midln:
                    nc.vector.tensor_mul(dst, t1[0:mw, :], rb[0:mw, :])
                else:
                    nc.vector.tensor_mul(t1[0:mw, :], t1[0:mw, :], rb[0:mw, :])
                    nc.vector.tensor_scalar(dst, t1[0:mw, :],
                                            midg_t[0:mw, col:col + 1],
                                            midb_t[0:mw, col:col + 1],
                                            ALU.mult, ALU.add)

            # q normalize (block-scoped), k normalize (in place, persistent)
            qhat = [bp.tile([128, BLK], BF16, tag=f"qh{c}", name=f"qh{c}_{b}",
                            bufs=1)
                    for c in range(2)]
            for c in range(2):
                mid_norm(qhat[c][:, :], qraw[c][:, :], c)
            for c in range(2):
                mid_norm(hk[c][:, i0:i0 + BLK], hk[c][:, i0:i0 + BLK], 2 + c)
            # p normalize (in place) + boundary
            for pc in range(8):
                mid_norm(praw[pc][:, :], praw[pc][:, :], 6 + pc)
            nc.vector.tensor_sub(pbord, pbord, mb[0:1, :])
            nc.vector.tensor_mul(pbord, pbord, rb[0:1, :])
            if not identity_midln:
                nc.vector.tensor_scalar(pbord, pbord,
                                        midg_t[0:1, NMC:NMC + 1],
                                        midb_t[0:1, NMC:NMC + 1],
                                        ALU.mult, ALU.add)
            # boundary rows for the roll-shift (pc 0 uses pbord directly)
            bnds = [pbord]
            for pc in range(1, 8):
                bnd = rowb_sl(8 + pc)
                nc.sync.dma_start(bnd, praw[pc - 1][127:128, :])
                bnds.append(bnd)

            cat_tiles = [catp.tile([128, BLK], BF16, tag=f"c{k}",
                                   name=f"c{k}_{b}")
                         for k in range(OKC)]

            # ---------- attention ----------
            LOOK = 2
            for s in range(HPC):
                off = 64 * (s % 2)
                q_ap = qhat[s // 2][off:off + D, :]
                k_tile = hk[s // 2]
                pav = pav_pool.tile([VST, BLK], F32, tag="av")
                kept = _kept_chunks(prune_dists[s], b)
                nk = len(kept)
                pps = [None] * nk
                c0s = [None] * nk
                for idx in range(nk + LOOK):
                    if idx < nk:
                        jc = kept[idx]
                        delta = JC * jc - i0
                        c0 = max(0, delta)
                        c0s[idx] = c0
                        pS = pmm.tile([128, BLK], F32, tag="mm")
                        nc.tensor.matmul(
                            pS[:, c0:BLK],
                            k_tile[off:off + D, JC * jc:JC * (jc + 1)],
                            q_ap[:, c0:BLK], start=True, stop=True)
                        nc.vector.tensor_add(pS[:, c0:BLK], pS[:, c0:BLK],
                                             tal8[s][:, c0:BLK])
                        if delta >= 0:
                            nc.vector.tensor_add(pS[:, delta:delta + JC],
                                                 pS[:, delta:delta + JC],
                                                 tri8[:, :])
                        pp = bp.tile([128, BLK], BF16, tag="pp", bufs=3)
                        nc.scalar.activation(
                            pp[:, c0:BLK], pS[:, c0:BLK], AF.Exp,
                            bias=ccbs[s][:, NJ * b + jc:NJ * b + jc + 1],
                            scale=scl8_t[:, 0:1])
                        pps[idx] = pp
                    av = idx - LOOK
                    if av >= 0:
                        jc = kept[av]
                        c0 = c0s[av]
                        nc.tensor.matmul(
                            pav[:, c0:BLK],
                            vhat[s][:, VST * jc:VST * jc + VST],
                            pps[av][:, c0:BLK],
                            start=(av == 0), stop=(av == nk - 1))
                        pps[av] = None
                # epilogue: o = (pav[0:64] - bcast(c-sum)) * bcast(1/z)
                zinv = rowb_sl(s)
                with nc.allow_low_precision(reason="softmax 1/z in bf16"):
                    nc.vector.reciprocal(zinv, pav[64:65, :])
                sac = rowb_sl(4 + s)
                nc.scalar.copy(sac, pav[96:97, :])
                pzb = pmm.tile([128, BLK], F32, tag="mm")
                nc.tensor.matmul(pzb[0:64, :], onesb2_t[0:1, 0:64],
                                 zinv, start=True, stop=True)
                nc.tensor.matmul(pzb[64:128, :], onesb2_t[0:1, 0:64],
                                 sac, start=True, stop=True)
                ct = cat_tiles[s // 2][off:off + D, :]
                nc.vector.tensor_copy(ct, pav[0:D, :])
                nc.vector.tensor_sub(ct, ct, pzb[64:128, :])
                nc.vector.tensor_mul(ct, ct, pzb[0:64, :])
                if not identity_midln:
                    nc.vector.tensor_scalar(ct, ct,
                                            midg_t[off:off + D, 4 + s // 2],
                                            midb_t[off:off + D, 4 + s // 2],
                                            ALU.mult, ALU.add)

            if STAGE < 4:
                return
            # ---------- pg roll-shift on PE + gelu gate multiply ----------
            for pc in range(8):
                gel = bp.tile([128, BLK], BF16, tag="gel")
                nc.scalar.activation(gel[:, :], praw[pc][:, :], AF.Gelu)
                psh = pmm.tile([128, BLK], F32, tag="mm")
                nc.tensor.matmul(psh[:, :], ishb_t[:, :], praw[pc][:, :],
                                 start=True, stop=False)
                nc.tensor.matmul(psh[:, :], id128_t[0:1, :], bnds[pc],
                                 start=False, stop=True)
                nc.vector.tensor_mul(cat_tiles[2 + pc][:, :], gel[:, :],
                                     psh[:, :])

            # ---------- out_proj ----------
            for tcn in range(BLK // 128):
                for nn in range(HID // BLK):
                    po = pmm.tile([128, BLK], F32, tag="mm")
                    for kc in range(OKC):
                        nc.tensor.matmul(
                            po[:, :],
                            cat_tiles[kc][:, 128 * tcn:128 * (tcn + 1)],
                            wo[kc][:, BLK * nn:BLK * (nn + 1)],
                            start=(kc == 0), stop=(kc == OKC - 1))
                    pos = bp.tile([128, BLK], BF16, tag="pos")
                    nc.vector.tensor_copy(pos[:, :], po[:, :])
                    nc.sync.dma_start(
                        po_in[b][128 * tcn:128 * (tcn + 1),
                                 BLK * nn:BLK * (nn + 1)], pos[:, :])
            nc.gpsimd.collective_compute(
                "ReduceScatter", ALU.add, replica_groups=REPLICA_GROUPS,
                ins=[po_in[b]], outs=[po_out[b]])

        def emit_fin(b):
            fl = big.tile([128, HID], BF16, tag="bfl", bufs=1)
            nc.sync.dma_start(fl[:, :], po_out[b])
            frow = big.tile([128, 8], F32, tag="bsrow")
            fsq = big.tile([128, HID], BF16, tag="bscr", bufs=1)
            nc.vector.tensor_reduce(frow[:, 0:1], fl[:, :], axis=AX.X,
                                    op=ALU.add)
            nc.vector.tensor_mul(fsq[:, :], fl[:, :], fl[:, :])
            nc.vector.tensor_reduce(frow[:, 1:2], fsq[:, :], axis=AX.X,
                                    op=ALU.add)
            ln_rows_col(frow, HID)
            fy = big.tile([128, HID], F32, tag="bx")
            nc.vector.tensor_scalar(fy[:, :], fl[:, :], frow[:, 6:7],
                                    frow[:, 7:8], ALU.mult, ALU.add)
            if not identity_outln:
                nc.vector.tensor_mul(fy[:, :], fy[:, :], og_bc[:, :])
                nc.vector.tensor_add(fy[:, :], fy[:, :], ob_bc[:, :])
            nc.sync.dma_start(y_out[128 * b:128 * (b + 1), :], fy[:, :])

        # ---------------- pipelined schedule ----------------
        if STAGE >= 2:
            for b in range(min(2, NBLK)):
                emit_inp(b)
            for b in range(NBLK):
                if STAGE >= 3:
                    emit_post(b)
                if b + 2 < NBLK:
                    emit_inp(b + 2)
                if STAGE >= 5 and b >= 1:
                    emit_fin(b - 1)
            if STAGE >= 5:
                emit_fin(NBLK - 1)

    return nc


# ---------------- host side ----------------

def _bf16(a):
    return np.asarray(a, dtype=np.float32).astype(ml_dtypes.bfloat16)


def prep_inputs(x, in_ln_g, in_ln_b, W_in, mid_ln_g, mid_ln_b, slopes,
                W_out, out_ln_g, out_ln_b, L):
    NBLK = L // BLK
    NJ = L // JC
    slopes = np.asarray(slopes, dtype=np.float32)
    order = np.argsort(-slopes, kind="stable")
    sorted_slopes = slopes[order]
    prune = _prune_dists(sorted_slopes, L)
    identity_outln = (np.allclose(out_ln_g, 1.0) and np.allclose(out_ln_b, 0.0))
    identity_inln = (np.allclose(in_ln_g, 1.0) and np.allclose(in_ln_b, 0.0))
    identity_midln = (np.allclose(mid_ln_g, 1.0)
                      and np.allclose(mid_ln_b, 0.0))

    tri8 = np.where(np.arange(JC)[:, None] > np.arange(JC)[None, :],
                    np.float32(NEG), np.float32(0.0)).astype(np.float32)
    eyeb = np.vstack([np.eye(D, dtype=np.float32)] * 2)
    id128 = np.eye(128, dtype=np.float32).astype(ml_dtypes.bfloat16)
    ishb = np.zeros((128, 128), np.float32)
    for p in range(127):
        ishb[p, p + 1] = 1.0
    ishb = ishb.astype(ml_dtypes.bfloat16)
    eyef = np.eye(128, dtype=np.float32)

    jr = np.arange(JC, dtype=np.float32)[:, None]
    ic = np.arange(BLK, dtype=np.float32)[None, :]

    in_maps = []
    meta = []
    for core in range(NCORES):
        bb, r = core // TP, core % TP
        heads = [int(order[TP * s + r]) for s in range(HPC)]
        rows = []
        for part in range(3):
            for h in heads:
                rows += list(range(part * HID + h * D, part * HID + (h + 1) * D))
        p0 = r * PW
        rows += list(range(3 * HID + p0, 3 * HID + p0 + PW))
        rows.append(3 * HID + (p0 - 1) % PFULL)
        rows = np.asarray(rows)
        cols = []
        for h in heads:
            cols += list(range(h * D, (h + 1) * D))
        cols += list(range(HID + p0, HID + p0 + PW))
        cols = np.asarray(cols)

        tal8 = np.stack([8.0 * slopes[h] * (jr - ic) for h in heads]
                        ).astype(np.float32)
        ccb = np.zeros((HPC, 128, NBLK * NJ), np.float32)
        for s, h in enumerate(heads):
            for b in range(NBLK):
                for jc in range(NJ):
                    delta = 128.0 * jc - BLK * b
                    ccb[s, :, NJ * b + jc] = slopes[h] * delta - C_SAFE

        in_maps.append({
            "x": np.ascontiguousarray(x[bb], dtype=np.float32),
            "w_inT": np.ascontiguousarray(_bf16(W_in[rows]).T),
            "w_outT": np.ascontiguousarray(_bf16(W_out[:, cols]).T),
            "mid_g": np.ascontiguousarray(mid_ln_g[rows]).astype(np.float32),
            "mid_b": np.ascontiguousarray(mid_ln_b[rows]).astype(np.float32),
            "in_g": np.asarray(in_ln_g, dtype=np.float32),
            "in_b": np.asarray(in_ln_b, dtype=np.float32),
            "out_g": np.asarray(out_ln_g, dtype=np.float32),
            "out_b": np.asarray(out_ln_b, dtype=np.float32),
            "tal8": tal8,
            "tri8": tri8,
            "ccb": ccb,
            "eyeb": eyeb,
            "id128": id128,
            "ishb": ishb,
            "eyef": eyef,
        })
        meta.append((bb, r))
    return in_maps, meta, prune, (identity_outln, identity_inln, identity_midln)


def unshard(results, meta, L):
    NBLK = L // BLK
    out = np.zeros((B, L, HID), np.float32)
    for core, (bb, r) in enumerate(meta):
        y = results[core]["y"]          # [L//TP, HID]
        for blk in range(NBLK):
            out[bb, BLK * blk + 128 * r: BLK * blk + 128 * r + 128, :] = \
                y[128 * blk:128 * (blk + 1), :]
    return out


def kernel(**inputs):
    L = inputs["x"].shape[1]
    in_maps, meta, prune, ident = prep_inputs(
        inputs["x"], inputs["in_ln_g"], inputs["in_ln_b"], inputs["W_in"],
        inputs["mid_ln_g"], inputs["mid_ln_b"], inputs["slopes"],
        inputs["W_out"], inputs["out_ln_g"], inputs["out_ln_b"], L)
    nc = build_program(L, prune, ident[0], ident[1], ident[2])
    nc.finalize()
    res = run_bass_kernel_spmd(nc, in_maps, list(range(NCORES)))
    global LAST_RESULT
    LAST_RESULT = res
    return unshard(res.results, meta, L)


if __name__ == "__main__":
    print("kernel module; use test.py")

